# revision 6
# baseline (speedup 1.0000x reference)
"""Trainium2 Bass kernel for a single decoder block (B=2, T=2048, C=1024,
NH=16, DFF=4096), distributed over 8 NeuronCores.

Sharding: token-parallel with STRIDED q-tile ownership for causal load
balance. Core c (member m = c%4 of batch c//4) owns the four 128-token
tiles {m, m+4, m+8, m+12} of its batch. Each core computes q/k/v for its
own tokens, AllGathers k and v within its 4-core batch group, then runs
causally-scheduled attention: key tile (g, u) (member g's u-th tile,
global tile 4u+g) is matched only against the suffix of owned q-tiles
j >= u; the first chunk of each suffix is masked per-core with a
ones/tri/zero 128x128 data mask (full/diagonal/invisible). This executes
40/64 of the dense score/AV work, identically on every core (SPMD-safe).

Attention output is accumulated token-major ([token, head*hd]) so the
softmax denominator sits on the free dim and normalization + LayerNorm
are cheap per-partition vector ops. FFN runs feature-major; the final
LayerNorm is token-major after a PE transpose.
"""

import sys

if "/opt/trn_rl_repo" not in sys.path:
    sys.path.insert(0, "/opt/trn_rl_repo")

from contextlib import ExitStack

import numpy as np
import ml_dtypes

B, T, C = 2, 2048, 1024
NH, HD, DFF = 16, 64, 4096
N_CORES = 8
TCH = 512          # tokens per core
NCT = C // 128     # 8 feature tiles
NPAIR = NH // 2    # 8 head pairs
NQT = 4            # owned 128-token q tiles per core
SCALE = 1.0 / 32.0  # 1/sqrt(C)
EPS = 1e-5

_CACHE = {}


def _owned_tiles(m):
    return [m, m + 4, m + 8, m + 12]


def _build(reps=1, collective=True):
    import concourse.mybir as mybir
    import concourse.tile as tile
    from concourse import bacc

    F32 = mybir.dt.float32
    BF16 = mybir.dt.bfloat16
    AF = mybir.ActivationFunctionType
    ALU = mybir.AluOpType

    nc = bacc.Bacc("TRN2", target_bir_lowering=False, debug=False,
                   num_devices=N_CORES)

    def din(name, shape):
        return nc.dram_tensor(name, shape, F32, kind="ExternalInput").ap()

    def dinb(name, shape):
        return nc.dram_tensor(name, shape, mybir.dt.bfloat16,
                              kind="ExternalInput").ap()

    xct = dinb("xct", [C, TCH])
    wq = dinb("wq", [C, C])
    wk = dinb("wk", [C, C])
    wv = dinb("wv", [C, C])
    w1 = dinb("w1", [C, DFF])
    w2 = dinb("w2", [DFF, C])
    bq_r = din("bq_r", [128, NCT])
    bk_r = din("bk_r", [128, NCT])
    bv_row = din("bv_row", [1, C])
    b1_r = din("b1_r", [128, DFF // 128])
    b2_r = din("b2_r", [128, NCT])
    lnw_row = din("lnw_row", [1, C])
    identb_in = dinb("identb_in", [128, 128])
    maskq = dinb("maskq", [4, 128, 128])
    outc = nc.dram_tensor("outc", [TCH, C], F32, kind="ExternalOutput").ap()

    import concourse.bass as bass

    def bcast_row(row_ap, parts=128):
        # view a [1, N] DRAM row as [parts, N] with partition step 0
        return bass.AP(tensor=row_ap.tensor, offset=row_ap.offset,
                       ap=[[0, parts]] + list(row_ap.ap[1:]))

    with tile.TileContext(nc) as tc, ExitStack() as S0, \
            nc.allow_low_precision(reason="bf16 matmul operand rounding"):
      persist = S0.enter_context(tc.tile_pool(name="persist", bufs=1))
      dram = S0.enter_context(tc.tile_pool(name="dram", bufs=1, space="DRAM"))
      for _rep in range(reps):

          # ---- phase-1-critical constants only; the rest load later
          bk_sb = persist.tile([128, NCT], F32)
          nc.sync.dma_start(bk_sb, bk_r)
          bq_sb = persist.tile([128, NCT], F32)
          nc.sync.dma_start(bq_sb, bq_r)
          eps_sb = persist.tile([128, 1], F32)
          nc.vector.memset(eps_sb, EPS)
          ident_b = persist.tile([128, 128], BF16)
          bv_sb = persist.tile([128, C], F32)
          lnwbc_sb = persist.tile([128, C], F32)
          b1_sb = persist.tile([128, DFF // 128], F32)
          b2_sb = persist.tile([128, NCT], F32)

          # ---- DRAM buffers for the k/v AllGathers (v first, k second)
          VROW = NH * (HD + 1)  # 1040: per-token v row with ones col per head
          kg_loc = dram.tile([C * TCH], BF16)
          vg_loc = dram.tile([TCH * VROW], BF16)
          kg_out = dram.tile([4, C * TCH], BF16)
          vg_out = dram.tile([4, TCH * VROW], BF16)
          k_loc = kg_loc[:].rearrange("(f t) -> f t", t=TCH)   # [1024, 512]
          v_loc = vg_loc[:].rearrange("(t x) -> t x", x=VROW)  # [512, 1040]

          with ExitStack() as SQA:
              qa_pool = SQA.enter_context(tc.tile_pool(name="qa", bufs=1))
              qT = qa_pool.tile([128, NPAIR, TCH], BF16)
              a_tm = qa_pool.tile([128, NQT, C], F32)  # token-major attn out
              w1p = SQA.enter_context(tc.tile_pool(name="w1p", bufs=3))
              w2p = SQA.enter_context(tc.tile_pool(name="w2p", bufs=2))
              SATT = SQA.enter_context(ExitStack())
              mpool = SATT.enter_context(tc.tile_pool(name="mpool", bufs=1))
              vaugp = SATT.enter_context(tc.tile_pool(name="vaugp", bufs=1))
              kpool = SATT.enter_context(tc.tile_pool(name="kpool", bufs=6))
              masks_sb = mpool.tile([128, 4, 128], BF16)

              # ================= Phase 1: x^T, q/k/v projections ============
              with ExitStack() as S1:
                  xtp = S1.enter_context(tc.tile_pool(name="xtp", bufs=1))
                  wqk = S1.enter_context(tc.tile_pool(name="wqk", bufs=3))
                  wvp = S1.enter_context(tc.tile_pool(name="wvp", bufs=2))
                  kvsb = S1.enter_context(tc.tile_pool(name="kvsb", bufs=4))
                  vstgp = S1.enter_context(tc.tile_pool(name="vstgp", bufs=1))
                  qkps = S1.enter_context(tc.tile_pool(name="qkps", bufs=4, space="PSUM"))
                  vps = S1.enter_context(tc.tile_pool(name="vps", bufs=4, space="PSUM"))

                  xT = xtp.tile([128, NCT, TCH], BF16)
                  nc.gpsimd.dma_start(
                      xT, xct.rearrange("(ci p) t -> p ci t", p=128))

                  # k^T first: it gates the k AllGather and scores
                  for p in range(NCT):
                      wt = wqk.tile([128, NCT, 128], BF16, tag="wt")
                      nc.sync.dma_start(
                          wt, wk.rearrange("(ci r) f -> r ci f", r=128)
                          [:, :, p * 128:(p + 1) * 128])
                      ps = qkps.tile([128, TCH], F32, tag="qkp")
                      for ci in range(NCT):
                          nc.tensor.matmul(ps, wt[:, ci, :], xT[:, ci, :],
                                           start=(ci == 0), stop=(ci == NCT - 1))
                      ksb = kvsb.tile([128, TCH], BF16, tag="ksb")
                      nc.scalar.activation(ksb, ps, AF.Identity,
                                           bias=bk_sb[:, p:p + 1])
                      nc.sync.dma_start(k_loc[p * 128:(p + 1) * 128, :], ksb)
                  if collective:
                      nc.gpsimd.collective_compute(
                          "AllGather", mybir.AluOpType.bypass,
                          replica_groups=[[0, 1, 2, 3], [4, 5, 6, 7]],
                          ins=[kg_loc[:].opt()], outs=[kg_out[:].opt()])
                  else:
                      for g in range(4):
                          nc.sync.dma_start(kg_out[g, 0:1024], kg_loc[0:1024])

                  nc.gpsimd.dma_start(bv_sb, bcast_row(bv_row))

                  # v: token-major, written directly in v_aug layout
                  # [512 tok, 16 head, 64+1] with a ones column per head
                  vstg = [vstgp.tile([128, NH, HD + 1], BF16, name=f"vstg{tt}")
                          for tt in range(4)]
                  for tt in range(4):
                      nc.vector.memset(vstg[tt][:, :, HD:HD + 1], 1.0)
                  for fvt in range(2):
                      wt = wvp.tile([128, NCT, TCH], BF16, tag="wtv")
                      nc.sync.dma_start(
                          wt, wv.rearrange("(ci r) f -> r ci f", r=128)
                          [:, :, fvt * TCH:(fvt + 1) * TCH])
                      pss = [vps.tile([128, TCH], F32, tag="vp", name=f"vp{fvt}_{i}")
                             for i in range(4)]
                      for ci in range(NCT):
                          for tt in range(4):
                              nc.tensor.matmul(
                                  pss[tt], xT[:, ci, tt * 128:(tt + 1) * 128],
                                  wt[:, ci, :],
                                  start=(ci == 0), stop=(ci == NCT - 1))
                      bvv = bv_sb[:, fvt * TCH:(fvt + 1) * TCH].rearrange(
                          "p (h d) -> p h d", d=HD)
                      for tt in range(4):
                          nc.vector.tensor_add(
                              vstg[tt][:, 8 * fvt:8 * fvt + 8, 0:HD],
                              pss[tt][:, :].rearrange("p (h d) -> p h d", d=HD),
                              bvv)
                  for tt in range(4):
                      nc.sync.dma_start(
                          v_loc[tt * 128:(tt + 1) * 128, :],
                          vstg[tt][:, :, :])
                  if collective:
                      nc.gpsimd.collective_compute(
                          "AllGather", mybir.AluOpType.bypass,
                          replica_groups=[[0, 1, 2, 3], [4, 5, 6, 7]],
                          ins=[vg_loc[:].opt()], outs=[vg_out[:].opt()])
                  else:
                      for g in range(4):
                          nc.sync.dma_start(vg_out[g, 0:1024], vg_loc[0:1024])

                  nc.sync.dma_start(
                      masks_sb, maskq.rearrange("g p t -> p g t"))
                  nc.sync.dma_start(ident_b, identb_in)
                  nc.gpsimd.dma_start(lnwbc_sb, bcast_row(lnw_row))
                  nc.sync.dma_start(b1_sb, b1_r)
                  nc.sync.dma_start(b2_sb, b2_r)

                  # q^T: [1024 feat, 512 tok], feature-major
                  for p in range(NCT):
                      wt = wqk.tile([128, NCT, 128], BF16, tag="wt")
                      nc.sync.dma_start(
                          wt, wq.rearrange("(ci r) f -> r ci f", r=128)
                          [:, :, p * 128:(p + 1) * 128])
                      ps = qkps.tile([128, TCH], F32, tag="qkp")
                      for ci in range(NCT):
                          nc.tensor.matmul(ps, wt[:, ci, :], xT[:, ci, :],
                                           start=(ci == 0), stop=(ci == NCT - 1))
                      nc.scalar.activation(qT[:, p, :], ps, AF.Identity,
                                           bias=bq_sb[:, p:p + 1])

              # ================= Phase 3: attention =========================
              with ExitStack() as S3:
                  pup = S3.enter_context(tc.tile_pool(name="pup", bufs=4))
                  dnp = S3.enter_context(tc.tile_pool(name="dnp", bufs=2))
                  stps = S3.enter_context(tc.tile_pool(name="stps", bufs=2, space="PSUM"))
                  avps = S3.enter_context(tc.tile_pool(name="avps", bufs=2, space="PSUM"))

                  # v_aug[token, slot(g,u)=4g+u, head, 0:64]=v, [.., 64]=1.0
                  vaug = vaugp.tile([128, 16, NH, HD + 1], BF16)
                  for g in range(4):
                      vsrc = vg_out[g].rearrange("(t x) -> t x", x=VROW)
                      for u in range(4):
                          nc.gpsimd.dma_start(
                              vaug[:, 4 * g + u, :, :].rearrange(
                                  "p h d -> p (h d)"),
                              vsrc[u * 128:(u + 1) * 128, :])

                  for p in range(NPAIR):
                      # av[token, 4*h + j, 0:65]: attn accum for head 2p+h,
                      # owned q-tile j; col 64 is the softmax denominator
                      av = avps.tile([128, 8, 128], F32, tag="av")
                      # 8 accumulation regions share 2 PSUM banks; a matmul
                      # start=True resets the whole bank, so zero via DVE and
                      # accumulate with start=False throughout.
                      nc.vector.memset(av[:, :, :], 0.0)
                      for g in range(4):
                          ktile = kpool.tile([128, TCH], BF16, tag="kt")
                          ksrc = kg_out[g].rearrange("(f t) -> f t", t=TCH)
                          nc.sync.dma_start(
                              ktile, ksrc[p * 128:(p + 1) * 128, :])
                          for u in range(4):
                              off = 128 * u
                              N = TCH - off
                              st = stps.tile([128, 2, TCH], F32, tag="st")
                              nc.tensor.matmul(
                                  st[:, 0, 0:N],
                                  ktile[0:64, u * 128:(u + 1) * 128],
                                  qT[0:64, p, off:TCH],
                                  start=True, stop=True)
                              nc.tensor.matmul(
                                  st[:, 1, 0:N],
                                  ktile[64:128, u * 128:(u + 1) * 128],
                                  qT[64:128, p, off:TCH],
                                  start=True, stop=True)
                              pu = pup.tile([128, 2, TCH], BF16, tag="pu")
                              nc.scalar.activation(pu[:, :, 0:N], st[:, :, 0:N],
                                                   AF.Exp, scale=SCALE)
                              m = masks_sb[:, g, :]
                              m2 = bass.AP(tensor=m.tensor, offset=m.offset,
                                           ap=[list(m.ap[0]), [0, 2],
                                               list(m.ap[1])])
                              nc.vector.tensor_mul(pu[:, :, 0:128],
                                                   pu[:, :, 0:128], m2)
                              for h in range(2):
                                  for j in range(u, 4):
                                      nc.tensor.matmul(
                                          av[:, 4 * h + j, 0:65],
                                          pu[:, h,
                                             (j - u) * 128:(j - u + 1) * 128],
                                          vaug[:, 4 * g + u, 2 * p + h, :],
                                          start=False,
                                          stop=(g == 3 and u == j),
                                          skip_group_check=True)
                      # normalize: a = av[0:64] / av[64], written token-major
                      rec = dnp.tile([128, 8, 1], F32, tag="rec")
                      nc.vector.reciprocal(rec[:, :, :], av[:, :, 64:65])
                      for h in range(2):
                          hc = (2 * p + h) * HD
                          for j in range(4):
                              nc.vector.tensor_scalar_mul(
                                  a_tm[:, j, hc:hc + HD],
                                  av[:, 4 * h + j, 0:64],
                                  rec[:, 4 * h + j, 0:1])

              SATT.close()

              # ====== Phase 4: h = a + LN(a), token-major; transpose to h^T
              with ExitStack() as SH:
                  hp = SH.enter_context(tc.tile_pool(name="hp", bufs=1))
                  hT = hp.tile([128, NCT, TCH], BF16)
                  h_tm = hp.tile([128, NQT, C], BF16)

                  with ExitStack() as S4:
                      sqp = S4.enter_context(tc.tile_pool(name="sqp", bufs=4))
                      tps4 = S4.enter_context(tc.tile_pool(name="tps4", bufs=4, space="PSUM"))

                      for j in range(NQT):
                          stats = sqp.tile([128, 2, nc.vector.BN_STATS_DIM],
                                           F32, tag="bst")
                          nc.vector.bn_stats(stats[:, 0], a_tm[:, j, 0:512])
                          nc.vector.bn_stats(stats[:, 1], a_tm[:, j, 512:1024])
                          mv = sqp.tile([128, nc.vector.BN_AGGR_DIM], F32,
                                        tag="mv")
                          nc.vector.bn_aggr(mv, stats)
                          rs = sqp.tile([128, 1], F32, tag="rs")
                          nc.scalar.activation(rs, mv[:, 1:2], AF.Sqrt,
                                               bias=eps_sb)
                          nc.vector.reciprocal(rs, rs)
                          t1 = sqp.tile([128, C], F32, tag="t1")
                          nc.vector.tensor_scalar(
                              out=t1, in0=a_tm[:, j, :], scalar1=mv[:, 0:1],
                              scalar2=rs, op0=ALU.subtract, op1=ALU.mult)
                          t2 = sqp.tile([128, C], F32, tag="t2")
                          nc.vector.tensor_mul(t2, t1, lnwbc_sb)
                          nc.vector.tensor_add(h_tm[:, j, :], t2, a_tm[:, j, :])
                          for ci in range(NCT):
                              tp = tps4.tile([128, 128], BF16, tag="tp")
                              nc.tensor.transpose(
                                  tp, h_tm[:, j, ci * 128:(ci + 1) * 128],
                                  ident_b)
                              nc.vector.tensor_copy(
                                  hT[:, ci, j * 128:(j + 1) * 128], tp)

                  # ================= Phase 5/6: FFN =========================
                  fT = hp.tile([128, NCT, TCH], BF16)
                  with ExitStack() as S5:
                      gp = S5.enter_context(tc.tile_pool(name="gp", bufs=1))
                      ffps = S5.enter_context(tc.tile_pool(name="ffps", bufs=4, space="PSUM"))

                      gT = gp.tile([128, DFF // 128, TCH], BF16)
                      for mt in range(DFF // 128):
                          wt = w1p.tile([128, NCT, 128], BF16, tag="w1t")
                          nc.sync.dma_start(
                              wt, w1.rearrange("(ci r) f -> r ci f", r=128)
                              [:, :, mt * 128:(mt + 1) * 128])
                          ps = ffps.tile([128, TCH], F32, tag="f1")
                          for ci in range(NCT):
                              nc.tensor.matmul(ps, wt[:, ci, :], hT[:, ci, :],
                                               start=(ci == 0), stop=(ci == NCT - 1))
                          nc.scalar.activation(gT[:, mt, :], ps, AF.Relu,
                                               bias=b1_sb[:, mt:mt + 1])
                      for ci in range(NCT):
                          wt = w2p.tile([128, DFF // 128, 128], BF16, tag="w2t")
                          nc.gpsimd.dma_start(
                              wt, w2.rearrange("(gk r) f -> r gk f", r=128)
                              [:, :, ci * 128:(ci + 1) * 128])
                          ps = ffps.tile([128, TCH], F32, tag="f2")
                          for gk in range(DFF // 128):
                              nc.tensor.matmul(ps, wt[:, gk, :], gT[:, gk, :],
                                               start=(gk == 0),
                                               stop=(gk == DFF // 128 - 1))
                          nc.scalar.activation(fT[:, ci, :], ps, AF.Identity,
                                               bias=b2_sb[:, ci:ci + 1])

                  # ================= Phase 7: out = f + LN(f), token-major ==
                  with ExitStack() as S7:
                      op7 = S7.enter_context(tc.tile_pool(name="op7", bufs=2))
                      tp7 = S7.enter_context(tc.tile_pool(name="tp7", bufs=4, space="PSUM"))

                      for tt in range(4):
                          ftok = op7.tile([128, C], F32, tag="ftok")
                          for ci in range(NCT):
                              tp = tp7.tile([128, 128], BF16, tag="tp")
                              nc.tensor.transpose(
                                  tp, fT[:, ci, tt * 128:(tt + 1) * 128],
                                  ident_b)
                              nc.vector.tensor_copy(
                                  ftok[:, ci * 128:(ci + 1) * 128], tp)
                          stats = op7.tile([128, 2, nc.vector.BN_STATS_DIM],
                                           F32, tag="bst")
                          nc.vector.bn_stats(stats[:, 0], ftok[:, 0:512])
                          nc.vector.bn_stats(stats[:, 1], ftok[:, 512:1024])
                          mv = op7.tile([128, nc.vector.BN_AGGR_DIM], F32,
                                        tag="mv")
                          nc.vector.bn_aggr(mv, stats)
                          rs7 = op7.tile([128, 1], F32, tag="rs7")
                          nc.scalar.activation(rs7, mv[:, 1:2], AF.Sqrt,
                                               bias=eps_sb)
                          nc.vector.reciprocal(rs7, rs7)
                          t1 = op7.tile([128, C], F32, tag="t17")
                          nc.vector.tensor_scalar(
                              out=t1, in0=ftok, scalar1=mv[:, 0:1], scalar2=rs7,
                              op0=ALU.subtract, op1=ALU.mult)
                          nc.vector.tensor_mul(t1, t1, lnwbc_sb)
                          otok = op7.tile([128, C], F32, tag="otok")
                          nc.vector.tensor_add(otok, t1, ftok)
                          nc.sync.dma_start(outc[tt * 128:(tt + 1) * 128, :],
                                            otok)

    nc.compile()
    return nc


def _stage(inputs):
    x = np.ascontiguousarray(np.asarray(inputs["x"], dtype=np.float32))
    bf = ml_dtypes.bfloat16
    shared = {
        "wq": np.ascontiguousarray(np.asarray(inputs["Wq"], np.float32).astype(bf)),
        "wk": np.ascontiguousarray(np.asarray(inputs["Wk"], np.float32).astype(bf)),
        "wv": np.ascontiguousarray(np.asarray(inputs["Wv"], np.float32).astype(bf)),
        "w1": np.ascontiguousarray(np.asarray(inputs["W1"], np.float32).astype(bf)),
        "w2": np.ascontiguousarray(np.asarray(inputs["W2"], np.float32).astype(bf)),
        "bq_r": np.ascontiguousarray(
            np.asarray(inputs["bq"], np.float32).reshape(NCT, 128).T),
        "bk_r": np.ascontiguousarray(
            np.asarray(inputs["bk"], np.float32).reshape(NCT, 128).T),
        "bv_row": np.ascontiguousarray(
            np.asarray(inputs["bv"], np.float32).reshape(1, C)),
        "b1_r": np.ascontiguousarray(
            np.asarray(inputs["b1"], np.float32).reshape(DFF // 128, 128).T),
        "b2_r": np.ascontiguousarray(
            np.asarray(inputs["b2"], np.float32).reshape(NCT, 128).T),
        "lnw_row": np.ascontiguousarray(
            np.asarray(inputs["ln_w"], np.float32).reshape(1, C)),
        "identb_in": np.eye(128, dtype=np.float32).astype(bf),
    }
    tri = (np.arange(128)[:, None] <= np.arange(128)[None, :])
    in_maps = []
    for c in range(N_CORES):
        b, m = divmod(c, 4)
        tok_idx = np.concatenate(
            [np.arange(128 * t, 128 * t + 128) for t in _owned_tiles(m)])
        mask = np.empty((4, 128, 128), dtype=np.float32)
        for g in range(4):
            if m > g:
                mask[g] = 1.0
            elif m == g:
                mask[g] = tri
            else:
                mask[g] = 0.0
        per = dict(shared)
        per["xct"] = np.ascontiguousarray(x[b, tok_idx, :].T.astype(bf))
        per["maskq"] = np.ascontiguousarray(mask.astype(bf))
        in_maps.append(per)
    return in_maps


def kernel(**inputs):
    from concourse.bass_utils import run_bass_kernel_spmd

    nc = _CACHE.get("nc")
    if nc is None:
        nc = _CACHE["nc"] = _build()
    in_maps = _stage(inputs)
    res = run_bass_kernel_spmd(nc, in_maps, core_ids=list(range(N_CORES)))
    out = np.empty((B, T, C), dtype=np.float32)
    for c in range(N_CORES):
        b, m = divmod(c, 4)
        tok_idx = np.concatenate(
            [np.arange(128 * t, 128 * t + 128) for t in _owned_tiles(m)])
        out[b, tok_idx, :] = res.results[c]["outc"]
    return out
